# revision 33
# baseline (speedup 1.0000x reference)
"""Trainium2 Bass kernel for bilinear forward-warp splatting (scatter_memory).

Computes, per batch element b (data-parallel over 8 NeuronCores):
    wy = y0 + dt*fy;  wx = x0 + dt*fx          (dt = tref - i)
    out[y, x] = sum_p v_p * tent(wy_p - y) * tent(wx_p - x)
for the three channels v in {1, fy, fx}, where tent(u) = max(0, 1-|u|) is
exactly the bilinear splat weight, followed by wf = splat(w*f)/(splat(w)+eps).

Algorithm (v2): displacement dt*flow is bounded (verified on host per region),
so the scatter is a windowed rank-1 accumulation. Points are chunked 128 at a
time as 16 rows x 8 comb-interleaved columns. Per chunk, a sparse y-tent
matrix (lhsT [128, mY]) and an x-tent-times-channels matrix (rhs [128, 3*XW])
are built on the DVE (custom fused relu(1-|a-b|) op); the TensorEngine
accumulates sum_p tentY[p,:]^T (x) rhs[p,:] into PSUM. Four 16-row bands
(one 64-row group) share one PSUM tile (per-band partition offsets), so PSUM
spills to the SBUF grid accumulators happen once per (group, column-segment).
Windows adapt per band (y) and per band-segment (x) from host-side maxima.
"""

import os
import sys
import math

import numpy as np

for _p in ("/opt/trn_rl_repo", "/root/.axon_site/_ro/trn_rl_repo"):
    if os.path.isdir(_p) and _p not in sys.path:
        sys.path.insert(0, _p)

from contextlib import ExitStack

import concourse.bass as bass
import concourse.bacc as bacc
import concourse.tile as tile
from concourse import mybir
from concourse.ap import AP
from concourse.bass_utils import run_bass_kernel_spmd

H, W = 480, 640
NCORES = 8
F32 = mybir.dt.float32
BF16 = mybir.dt.bfloat16  # bf16: f32 exponent range (tent weights down to ~1e-9 must not flush to 0)
Alu = mybir.AluOpType
Act = mybir.ActivationFunctionType

BH = 16          # band height (rows per band)
IL = 8           # column interleave; chunks are BH rows x IL cols = 128 points
GB = 4           # bands per PSUM group (GB*BH = 64-row groups; all bands at
                 # psum partition 0 — PE psum writes starting at 32 are
                 # limited to 32 partitions by the BIR verifier. The snapped
                 # prefix columns of each band's lhsT are zero-filled by the
                 # ACT engine so the DVE only evaluates the natural window.)
SP = 15          # chunk-columns (of IL cols) per PSUM segment
EPS = 1e-9
BIG = 4.0e6      # pushed onto wy for masked-out points -> tent == 0 everywhere

NBLK = (H + 127) // 128
NPAIRS = W // IL
NSEGS = (NPAIRS + SP - 1) // SP

_TENT_OP = None


def _tent_op():
    """Register (once) the fused tent op: out = relu(1 - |in0 - in1|)."""
    global _TENT_OP
    if _TENT_OP is not None:
        return _TENT_OP
    from concourse import dve_ops as dvo
    from concourse.dve_spec import Spec, Src0, Src1, One, maxx, relu, lower
    from concourse.dve_uop import DveOpSpec

    name = "TENT_ANT"
    for op in dvo.OPS:
        if op.name == name:
            _TENT_OP = op
            return op
    spec = Spec(
        body=relu(One - maxx(Src0 - Src1, Src1 - Src0)),
        reference=lambda in0, in1, s0, s1, imm2: np.maximum(
            0.0, 1.0 - np.abs(in0 - in1)
        ),
    )
    row = dvo._CUSTOM_DVE_ROW_BASE + len(dvo.OPS)
    shas = {}
    for ver in ("v3", "v4"):
        shas[ver] = DveOpSpec(
            name=name, opcode=row, uops=lower(spec, ver=ver), rd1_en=True
        ).sha(ver)
    op = dvo.DveOp(name, spec, subdim=False, uops_sha=shas)
    dvo.OPS.append(op)
    dvo._SUB_OPCODE_FOR_NAME[name] = row
    dvo.CUSTOM_DVE_SPECS[name] = spec
    _TENT_OP = op
    return op


def _v(ap, dims, extra_off=0, parts=None):
    """Manual AP view: keep ap's partition pair, replace free dims."""
    ppair = [ap.ap[0][0], ap.ap[0][1] if parts is None else parts]
    return AP(tensor=ap.tensor, offset=ap.offset + extra_off, ap=[ppair] + [list(d) for d in dims])


def _build_program(dt, disp_band, disp_grp, dx_map, H=H, W=W):
    """disp_band[30]: per-band y half-window; disp_grp[8]: per-64-row-group
    y half-window; dx_map[band][seg]: x half-window per region."""
    TENT = _tent_op()
    nbands = H // BH
    ngroups = (nbands + GB - 1) // GB
    assert len(disp_band) == nbands and len(disp_grp) == ngroups
    assert all(len(r) == NSEGS for r in dx_map)
    PAD = max(disp_grp) + 1
    assert PAD <= 32, "grid storage shift assumes PAD <= 32"
    NSBLK = (H + PAD + 127) // 128
    dxmax = max(max(r) for r in dx_map)
    XPAD = dxmax + 1
    # all bands write psum from partition 0; lhsT columns before the band's
    # natural window are zero (ACT-filled), so psum rows [0, prefix) get +0
    mYmax = max(BH * (bi % GB + 1) + disp_band[bi] + disp_grp[bi // GB] + 2
                for bi in range(len(disp_band)))
    XWMAX = 2 * dxmax + 2 + IL
    XT3MAX = (IL * SP + 2 * dxmax + 2) * 3   # psum extent (ch-inner) of a segment
    assert XT3MAX <= 512
    GRMAX = GB * BH + 2 * max(disp_grp) + 2  # psum rows of a group tile
    assert GRMAX <= 128

    nc = bacc.Bacc("TRN2", target_bir_lowering=False, debug=False)
    fy_in = nc.declare_dram_parameter("fy", [H, W], F32, isOutput=False)
    fx_in = nc.declare_dram_parameter("fx", [H, W], F32, isOutput=False)
    o_wfx = nc.declare_dram_parameter("out_wfx", [H, W], F32, isOutput=True)
    o_wfy = nc.declare_dram_parameter("out_wfy", [H, W], F32, isOutput=True)

    with ExitStack() as ctx:
        tc = ctx.enter_context(tile.TileContext(nc))
        singles = ctx.enter_context(tc.tile_pool(name="singles", bufs=1))

        # ---- constant ramps (f32 iotas: all values exact below 2^24) ----
        NY = H + 2 * PAD + 8
        NX = W + XWMAX + 8
        ioY = singles.tile([128, NY], F32)
        ioX = singles.tile([128, NX], F32)
        x0f = singles.tile([128, W], F32)
        y0f = singles.tile([128, NBLK], F32)
        nc.gpsimd.iota(ioY[:], pattern=[[1, NY]], base=-PAD, channel_multiplier=0,
                       allow_small_or_imprecise_dtypes=True)
        nc.gpsimd.iota(ioX[:], pattern=[[1, NX]], base=-(dxmax + 1), channel_multiplier=0,
                       allow_small_or_imprecise_dtypes=True)
        nc.gpsimd.iota(x0f[:], pattern=[[1, W]], base=0, channel_multiplier=0,
                       allow_small_or_imprecise_dtypes=True)
        nc.gpsimd.iota(y0f[:], pattern=[[128, NBLK]], base=0, channel_multiplier=1,
                       allow_small_or_imprecise_dtypes=True)

        # ---- grid accumulators (ch-inner: 0=w, 1=w*fy, 2=w*fx) ----
        # storage row = real + PAD; storage col = real + XPAD (halo cols/rows
        # only ever accumulate exact zeros)
        WS = W + 2 * XPAD
        grid = singles.tile([128, NSBLK, WS, 3], F32)
        nc.vector.memset(grid[:, :, :WS // 4], 0.0)
        nc.gpsimd.memset(grid[:, :, WS // 4:], 0.0)

        # zero operands for the per-group-segment PSUM-clearing matmul
        z_l = singles.tile([16, 128], BF16)
        z_r = singles.tile([16, 512], BF16)
        zbf = singles.tile([128, 1], BF16)
        cH = singles.tile([128, 1], F32)
        cW = singles.tile([128, 1], F32)
        nc.gpsimd.memset(z_l[:], 0.0)
        nc.gpsimd.memset(z_r[:], 0.0)
        nc.gpsimd.memset(zbf[:], 0.0)
        nc.gpsimd.memset(cH[:], float(H - 1))
        nc.gpsimd.memset(cW[:], float(W - 1))

        # ---- load inputs & pointwise prep (comb-split planes) ----
        # PSc: plane 0 = wyM (masked warped y), plane 1 = wx   (f32)
        # PSv: plane 0 = fy, plane 1 = fx                      (bf16)
        PSc = singles.tile([128, 2, NBLK, IL, NPAIRS], F32)
        PSv = singles.tile([128, 2, NBLK, IL, NPAIRS], BF16)

        with tc.tile_pool(name="inpool", bufs=1) as inpool, \
             tc.tile_pool(name="preptmp", bufs=2) as preptmp:
            in_fy = inpool.tile([128, NBLK, W], F32)
            in_fx = inpool.tile([128, NBLK, W], F32)
            for blk in range(NBLK):
                rows = min(128, H - 128 * blk)
                nc.gpsimd.dma_start(out=in_fy[:rows, blk], in_=fy_in.ap()[128 * blk:128 * blk + rows])
                nc.sync.dma_start(out=in_fx[:rows, blk], in_=fx_in.ap()[128 * blk:128 * blk + rows])
            for blk in range(NBLK):
                rows = min(128, H - 128 * blk)
                wy = preptmp.tile([128, W], F32, tag="wy")
                wx = preptmp.tile([128, W], F32, tag="wx")
                ta = preptmp.tile([128, W], F32, tag="ta")
                tb = preptmp.tile([128, W], F32, tag="tb")
                nc.vector.tensor_scalar(out=wy[:rows], in0=in_fy[:rows, blk], scalar1=dt,
                                        scalar2=y0f[:rows, blk:blk + 1], op0=Alu.mult, op1=Alu.add)
                nc.vector.scalar_tensor_tensor(out=wx[:rows], in0=in_fx[:rows, blk], scalar=dt,
                                               in1=x0f[:rows], op0=Alu.mult, op1=Alu.add)
                # mask: outside iff wy != clamp(wy) or wx != clamp(wx); exact 0/1 flags
                nc.vector.tensor_scalar(out=ta[:rows], in0=wy[:rows], scalar1=0.0,
                                        scalar2=float(H - 1), op0=Alu.max, op1=Alu.min)
                nc.vector.tensor_tensor(out=ta[:rows], in0=wy[:rows], in1=ta[:rows], op=Alu.not_equal)
                nc.vector.tensor_scalar(out=tb[:rows], in0=wx[:rows], scalar1=0.0,
                                        scalar2=float(W - 1), op0=Alu.max, op1=Alu.min)
                nc.vector.tensor_tensor(out=tb[:rows], in0=wx[:rows], in1=tb[:rows], op=Alu.not_equal)
                nc.vector.tensor_tensor(out=ta[:rows], in0=ta[:rows], in1=tb[:rows], op=Alu.add)
                # comb-split writes: dest view [rows, NPAIRS, IL] iterated (j, i), i fastest
                def split_view(t4, pl):
                    return t4[:rows, pl, blk].rearrange("p i j -> p j i")
                nc.vector.scalar_tensor_tensor(out=split_view(PSc, 0), in0=ta[:rows], scalar=BIG,
                                               in1=wy[:rows], op0=Alu.mult, op1=Alu.add)
                nc.scalar.activation(out=split_view(PSc, 1), in_=wx[:rows], func=Act.Copy)
                nc.scalar.activation(out=split_view(PSv, 0), in_=in_fy[:rows, blk], func=Act.Copy)
                nc.scalar.activation(out=split_view(PSv, 1), in_=in_fx[:rows, blk], func=Act.Copy)

        # ---- main banded splat ----
        bandp = ctx.enter_context(tc.tile_pool(name="bandp", bufs=3))
        build = ctx.enter_context(tc.tile_pool(name="build", bufs=3))
        psump = ctx.enter_context(tc.tile_pool(name="psump", bufs=1, space="PSUM"))
        tailp = ctx.enter_context(tc.tile_pool(name="tailp", bufs=2))

        for g in range(ngroups):
            gb_g = min(GB, nbands - g * GB)      # bands in this (maybe ragged) group
            ag = g * GB * BH                     # group start row
            Dg = disp_grp[g]
            ws_g = ag - (Dg + 1)                 # group window start (real row)
            rows_g = gb_g * BH + 2 * Dg + 2
            # one 6-bank psum tile holds all segments of the group; segment s
            # lives in bank s at cols [0, XT3MAX) with the uniform mapping
            # psum col 3*xi + c  <->  storage col IL*SP*s + xi
            ptile = psump.tile([128, NSEGS, 512], F32, tag="pseg", name=f"pseg_{g}")
            cleared = [False] * NSEGS

            for b in range(gb_g):
                bi = g * GB + b
                a = ag + b * BH
                Db = disp_band[bi]
                wstart = ws_g                    # real row of psum row 0
                mY = (a + BH + Db + 1) - wstart
                pfx = (a - Db - 1) - wstart      # zero lhsT cols before the window
                assert 0 <= pfx < mY <= mYmax and mY <= rows_g
                blk, p0 = divmod(a, 128)
                bandC = bandp.tile([128, 2, NPAIRS], F32, tag="bandC")   # wyM, wx
                bandV = bandp.tile([128, 2, NPAIRS], BF16, tag="bandV")  # fy, fx
                for i in range(IL):
                    nc.gpsimd.dma_start(out=bandC[BH * i:BH * (i + 1)],
                                        in_=PSc[p0:p0 + BH, :, blk, i])
                    nc.sync.dma_start(out=bandV[BH * i:BH * (i + 1)],
                                      in_=PSv[p0:p0 + BH, :, blk, i])
                for s in range(NSEGS):
                    SPs = min(SP, NPAIRS - SP * s)
                    dx = dx_map[bi][s]
                    delta = dxmax - dx   # column shift into the uniform mapping
                    XW = 2 * dx + 2 + IL
                    j0 = SP * s

                    tentY = build.tile([128, SP, mYmax], BF16, tag="tentY")
                    rhs = build.tile([128, SP, 3, XWMAX], BF16, tag="rhs")
                    fvE = build.tile([128, SP, 2, XWMAX], BF16, tag="fvE")

                    # Y tents: tentY = relu(1 - |ioY - wy|), one fused DVE pass
                    # over the natural window; ACT zero-fills the snap prefix
                    if pfx > 0:
                        nc.scalar.activation(
                            out=tentY[:, :SPs, :pfx],
                            in_=_v(zbf[:, 0:1], [[0, SPs], [0, pfx]]),
                            func=Act.Copy)
                    nc.vector._custom_dve(
                        TENT,
                        out=tentY[:, :SPs, pfx:mY],
                        in0=_v(ioY[:, (a - Db - 1) + PAD:], [[0, SPs], [1, mY - pfx]]),
                        in1=_v(bandC[:, 0, j0:j0 + SPs], [[1, SPs], [0, mY - pfx]]))
                    # X tents into rhs channel 0 (contiguous)
                    nc.vector._custom_dve(
                        TENT,
                        out=rhs[:, :SPs, 0, :XW],
                        in0=_v(ioX[:, IL * j0 + dxmax - dx:], [[IL, SPs], [1, XW]]),
                        in1=_v(bandC[:, 1, j0:j0 + SPs], [[1, SPs], [0, XW]]))
                    # expand fy/fx on the ACT engine so the DVE mul sees
                    # packed operands (keeps the 2x 16-bit DVE mode)
                    nc.scalar.activation(
                        out=fvE[:, :SPs, :, :XW],
                        in_=_v(bandV[:, 0, j0:j0 + SPs], [[1, SPs], [NPAIRS, 2], [0, XW]]),
                        func=Act.Copy)
                    # both value channels in one packed mul: ch0 read twice via
                    # a stride-0 middle dim (inner dim stays packed)
                    nc.vector.tensor_tensor(
                        out=rhs[:, :SPs, 1:3, :XW],
                        in0=_v(rhs[:, :SPs, 0, :XW], [[3 * XWMAX, SPs], [0, 2], [1, XW]]),
                        in1=fvE[:, :SPs, :, :XW],
                        op=Alu.mult)

                    if not cleared[s]:
                        # start=True zero matmul: clears the bank's has_written
                        # bits over the segment's full spill extent
                        XTg = IL * SPs + 2 * dxmax + 2
                        nc.tensor.matmul(ptile[:rows_g, s, :XTg * 3], lhsT=z_l[:, :rows_g],
                                         rhs=z_r[:, :XTg * 3], start=True, stop=False)
                        cleared[s] = True
                    last = (b == gb_g - 1)
                    for jj in range(SPs):
                        # rhs chunk read ch-inner (x outer, ch inner) to match psum
                        rhs_j = _v(rhs[:], [[1, XW], [XWMAX, 3]],
                                   extra_off=jj * 3 * XWMAX)
                        col0 = 3 * (IL * jj + delta)
                        nc.tensor.matmul(
                            ptile[0:mY, s, col0:col0 + XW * 3],
                            lhsT=tentY[:, jj, :mY],
                            rhs=rhs_j,
                            start=False, stop=(last and jj == SPs - 1))

            # spill: add group psum into the grid. Segments with the same
            # parity have disjoint storage-column windows, so each partition
            # piece needs one DVE add per parity class (plus the ragged seg).
            # storage col of psum (s, 3*xi+c) is IL*SP*s + xi (uniform map).
            y = ws_g + PAD            # storage row of psum row 0 (= ag, 32-aligned)
            assert y % 32 == 0
            s1 = min(y + rows_g, H + PAD)  # rows beyond H+PAD are exact zeros
            # engine partition-range rule: start q allows to the next
            # 64-boundary, except q%128==0 which allows all 128
            allow = lambda q: 128 - q % 128 if q % 128 == 0 else 64 - q % 64 if q % 64 else 64
            while y < s1:
                gblk, gp = divmod(y, 128)
                pr = y - (ws_g + PAD)
                ln = min(s1 - y, allow(gp), allow(pr))
                for s in range(NSEGS):
                    SPs = min(SP, NPAIRS - SP * s)
                    ext = (IL * SPs + 2 * dxmax + 2) * 3
                    c0 = IL * SP * s
                    nc.vector.tensor_tensor(
                        out=grid[gp:gp + ln, gblk, c0:c0 + ext // 3],
                        in0=_v(ptile[pr:pr + ln], [[1, ext]], extra_off=s * 512),
                        in1=grid[gp:gp + ln, gblk, c0:c0 + ext // 3],
                        op=Alu.add)
                y += ln

            # per-block tail: normalize + store any 128-row storage block that
            # no later group touches (overlaps the remaining main loop)
            for bb_ in range(NSBLK):
                if g == min(ngroups - 1, (128 * (bb_ + 1) - 1) // (GB * BH)):
                    rec = tailp.tile([128, W], F32, tag="rec", name=f"rec{bb_}")
                    ofy = tailp.tile([128, W], F32, tag="ofy", name=f"ofy{bb_}")
                    ofx = tailp.tile([128, W], F32, tag="ofx", name=f"ofx{bb_}")
                    nc.vector.tensor_scalar(out=rec[:], in0=grid[:, bb_, XPAD:XPAD + W, 0],
                                            scalar1=EPS, scalar2=None, op0=Alu.add)
                    nc.vector.reciprocal(out=rec[:], in_=rec[:])
                    nc.vector.tensor_tensor(out=ofy[:], in0=grid[:, bb_, XPAD:XPAD + W, 1],
                                            in1=rec[:], op=Alu.mult)
                    nc.gpsimd.tensor_tensor(out=ofx[:], in0=grid[:, bb_, XPAD:XPAD + W, 2],
                                            in1=rec[:], op=Alu.mult)
                    r0 = max(0, 128 * bb_ - PAD)
                    r1 = min(H, 128 * (bb_ + 1) - PAD)
                    p0_ = r0 + PAD - 128 * bb_
                    nc.sync.dma_start(out=o_wfx.ap()[r0:r1], in_=ofx[p0_:p0_ + r1 - r0])
                    nc.gpsimd.dma_start(out=o_wfy.ap()[r0:r1], in_=ofy[p0_:p0_ + r1 - r0])


    nc.compile()
    return nc


_PROG_CACHE = {}


def _get_program(dt, disp_band, disp_grp, dx_map, H=H, W=W):
    key = (float(dt), tuple(disp_band), tuple(disp_grp),
           tuple(tuple(r) for r in dx_map), H, W)
    if key not in _PROG_CACHE:
        _PROG_CACHE[key] = _build_program(dt, disp_band, disp_grp, dx_map, H=H, W=W)
    return _PROG_CACHE[key]


def _window_params(fy, fx, dt, H=H, W=W):
    """Exact per-region displacement bounds (over all batch elements)."""
    ady = np.abs(dt) * np.abs(fy).max(axis=0)      # [H, W]
    adx = np.abs(dt) * np.abs(fx).max(axis=0)
    nbands = H // BH
    ngroups = (nbands + GB - 1) // GB
    disp_band = [max(2, int(math.ceil(float(ady[a:a + BH].max()))))
                 for a in range(0, H, BH)]
    # all groups share the global max so the group window start lands exactly
    # PAD rows early -> storage partition of psum row 0 is 32-aligned (= a_g)
    dmax = max(disp_band)
    disp_grp = [dmax for _ in range(ngroups)]
    assert len(disp_grp) == ngroups
    dx_map = []
    for a in range(0, H, BH):
        row = []
        for s in range(NSEGS):
            c0 = IL * SP * s
            c1 = min(W, IL * SP * (s + 1))
            m = float(adx[a:a + BH, c0:c1].max())
            row.append(max(2, int(math.ceil(m))))
        dx_map.append(row)
    return disp_band, disp_grp, dx_map


def kernel(flow_maps_x, flow_maps_y, i=0, tref=4):
    i = int(i)
    tref = int(tref)
    dt = float(tref - i)
    B = flow_maps_x.shape[0]
    assert B <= NCORES, f"batch {B} > {NCORES} cores not supported"
    fx = np.ascontiguousarray(flow_maps_x[:, i]).astype(np.float32)
    fy = np.ascontiguousarray(flow_maps_y[:, i]).astype(np.float32)

    disp_band, disp_grp, dx_map = _window_params(fy, fx, dt)
    nc = _get_program(dt, disp_band, disp_grp, dx_map)
    in_maps = [{"fy": fy[b], "fx": fx[b]} for b in range(B)]
    res = run_bass_kernel_spmd(nc, in_maps, list(range(B)))
    wfx = np.stack([res.results[b]["out_wfx"] for b in range(B)])[:, None]
    wfy = np.stack([res.results[b]["out_wfy"] for b in range(B)])[:, None]
    return wfx.astype(np.float32), wfy.astype(np.float32)


def _ensure_ntff_hook():
    """The agent image lacks antenv.axon_hooks; synthesize it from trn_agent_boot."""
    import types
    try:
        import antenv.axon_hooks  # noqa: F401
        return
    except ImportError:
        pass
    from trn_agent_boot.trn_boot import _ntff_profile_via_ctypes
    hook = _ntff_profile_via_ctypes("/opt/axon/libaxon_pjrt.so")
    m = types.ModuleType("antenv.axon_hooks")
    m.get_axon_ntff_profile_hook = lambda: hook
    m.set_axon_ntff_profile_hook = lambda h: None
    sys.modules["antenv.axon_hooks"] = m


def timed_run(np_inputs):
    """Run once with NTFF tracing; return HW exec time in ns (max over traced cores)."""
    _ensure_ntff_hook()
    i = int(np_inputs["i"]); tref = int(np_inputs["tref"])
    dt = float(tref - i)
    fx = np.ascontiguousarray(np_inputs["flow_maps_x"][:, i]).astype(np.float32)
    fy = np.ascontiguousarray(np_inputs["flow_maps_y"][:, i]).astype(np.float32)
    B = fx.shape[0]
    disp_band, disp_grp, dx_map = _window_params(fy, fx, dt)
    nc = _get_program(dt, disp_band, disp_grp, dx_map)
    in_maps = [{"fy": fy[b], "fx": fx[b]} for b in range(B)]
    res = run_bass_kernel_spmd(nc, in_maps, list(range(B)), trace=True)
    return res.exec_time_ns


if __name__ == "__main__":
    rng = np.random.default_rng(0)
    fmx = rng.standard_normal((8, 4, H, W), dtype=np.float32)
    fmy = rng.standard_normal((8, 4, H, W), dtype=np.float32)
    ox, oy = kernel(fmx, fmy, 0, 4)
    print(ox.shape, oy.shape, ox.dtype)


# revision 34
# speedup vs baseline: 1.0805x; 1.0805x over previous
"""Trainium2 Bass kernel for bilinear forward-warp splatting (scatter_memory).

Computes, per batch element b (data-parallel over 8 NeuronCores):
    wy = y0 + dt*fy;  wx = x0 + dt*fx          (dt = tref - i)
    out[y, x] = sum_p v_p * tent(wy_p - y) * tent(wx_p - x)
for the three channels v in {1, fy, fx}, where tent(u) = max(0, 1-|u|) is
exactly the bilinear splat weight, followed by wf = splat(w*f)/(splat(w)+eps).

Algorithm (v2): displacement dt*flow is bounded (verified on host per region),
so the scatter is a windowed rank-1 accumulation. Points are chunked 128 at a
time as 16 rows x 8 comb-interleaved columns. Per chunk, a sparse y-tent
matrix (lhsT [128, mY]) and an x-tent-times-channels matrix (rhs [128, 3*XW])
are built on the DVE (custom fused relu(1-|a-b|) op); the TensorEngine
accumulates sum_p tentY[p,:]^T (x) rhs[p,:] into PSUM. Four 16-row bands
(one 64-row group) share one PSUM tile (per-band partition offsets), so PSUM
spills to the SBUF grid accumulators happen once per (group, column-segment).
Windows adapt per band (y) and per band-segment (x) from host-side maxima.
"""

import os
import sys
import math

import numpy as np

for _p in ("/opt/trn_rl_repo", "/root/.axon_site/_ro/trn_rl_repo"):
    if os.path.isdir(_p) and _p not in sys.path:
        sys.path.insert(0, _p)

from contextlib import ExitStack

import concourse.bass as bass
import concourse.bacc as bacc
import concourse.tile as tile
from concourse import mybir
from concourse.ap import AP
from concourse.bass_utils import run_bass_kernel_spmd

H, W = 480, 640
NCORES = 8
F32 = mybir.dt.float32
BF16 = mybir.dt.bfloat16  # bf16: f32 exponent range (tent weights down to ~1e-9 must not flush to 0)
Alu = mybir.AluOpType
Act = mybir.ActivationFunctionType

BH = 16          # band height (rows per band)
IL = 8           # column interleave; chunks are BH rows x IL cols = 128 points
GB = 4           # bands per PSUM group (GB*BH = 64-row groups; all bands at
                 # psum partition 0 — PE psum writes starting at 32 are
                 # limited to 32 partitions by the BIR verifier. The snapped
                 # prefix columns of each band's lhsT are zero-filled by the
                 # ACT engine so the DVE only evaluates the natural window.)
SP = 15          # chunk-columns (of IL cols) per PSUM segment
EPS = 1e-9
BIG = 4.0e6      # pushed onto wy for masked-out points -> tent == 0 everywhere

NBLK = (H + 127) // 128
NPAIRS = W // IL
NSEGS = (NPAIRS + SP - 1) // SP

_TENT_OP = None


def _tent_op():
    """Register (once) the fused tent op: out = relu(1 - |in0 - in1|)."""
    global _TENT_OP
    if _TENT_OP is not None:
        return _TENT_OP
    from concourse import dve_ops as dvo
    from concourse.dve_spec import Spec, Src0, Src1, One, maxx, relu, lower
    from concourse.dve_uop import DveOpSpec

    name = "TENT_ANT"
    for op in dvo.OPS:
        if op.name == name:
            _TENT_OP = op
            return op
    spec = Spec(
        body=relu(One - maxx(Src0 - Src1, Src1 - Src0)),
        reference=lambda in0, in1, s0, s1, imm2: np.maximum(
            0.0, 1.0 - np.abs(in0 - in1)
        ),
    )
    row = dvo._CUSTOM_DVE_ROW_BASE + len(dvo.OPS)
    shas = {}
    for ver in ("v3", "v4"):
        shas[ver] = DveOpSpec(
            name=name, opcode=row, uops=lower(spec, ver=ver), rd1_en=True
        ).sha(ver)
    op = dvo.DveOp(name, spec, subdim=False, uops_sha=shas)
    dvo.OPS.append(op)
    dvo._SUB_OPCODE_FOR_NAME[name] = row
    dvo.CUSTOM_DVE_SPECS[name] = spec
    _TENT_OP = op
    return op


def _v(ap, dims, extra_off=0, parts=None):
    """Manual AP view: keep ap's partition pair, replace free dims."""
    ppair = [ap.ap[0][0], ap.ap[0][1] if parts is None else parts]
    return AP(tensor=ap.tensor, offset=ap.offset + extra_off, ap=[ppair] + [list(d) for d in dims])


def _build_program(dt, disp_band, disp_grp, dx_map, H=H, W=W):
    """disp_band[30]: per-band y half-window; disp_grp[8]: per-64-row-group
    y half-window; dx_map[band][seg]: x half-window per region."""
    TENT = _tent_op()
    nbands = H // BH
    ngroups = (nbands + GB - 1) // GB
    assert len(disp_band) == nbands and len(disp_grp) == ngroups
    assert all(len(r) == NSEGS for r in dx_map)
    PAD = max(disp_grp) + 1
    assert PAD <= 32, "grid storage shift assumes PAD <= 32"
    NSBLK = (H + PAD + 127) // 128
    dxmax = max(max(r) for r in dx_map)
    XPAD = dxmax + 1
    # all bands write psum from partition 0; lhsT columns before the band's
    # natural window are zero (ACT-filled), so psum rows [0, prefix) get +0
    mYmax = max(BH * (bi % GB + 1) + disp_band[bi] + disp_grp[bi // GB] + 2
                for bi in range(len(disp_band)))
    XWMAX = 2 * dxmax + 2 + IL
    XT3MAX = (IL * SP + 2 * dxmax + 2) * 3   # psum extent (ch-inner) of a segment
    assert XT3MAX <= 512
    GRMAX = GB * BH + 2 * max(disp_grp) + 2  # psum rows of a group tile
    assert GRMAX <= 128

    nc = bacc.Bacc("TRN2", target_bir_lowering=False, debug=False)
    fy_in = nc.declare_dram_parameter("fy", [H, W], F32, isOutput=False)
    fx_in = nc.declare_dram_parameter("fx", [H, W], F32, isOutput=False)
    o_wfx = nc.declare_dram_parameter("out_wfx", [H, W], F32, isOutput=True)
    o_wfy = nc.declare_dram_parameter("out_wfy", [H, W], F32, isOutput=True)

    with ExitStack() as ctx:
        tc = ctx.enter_context(tile.TileContext(nc))
        singles = ctx.enter_context(tc.tile_pool(name="singles", bufs=1))

        # ---- constant ramps (f32 iotas: all values exact below 2^24) ----
        NY = H + 2 * PAD + 8
        NX = W + XWMAX + 8
        ioY = singles.tile([128, NY], F32)
        ioX = singles.tile([128, NX], F32)
        x0f = singles.tile([128, W], F32)
        y0f = singles.tile([128, NBLK], F32)
        nc.gpsimd.iota(ioY[:], pattern=[[1, NY]], base=-PAD, channel_multiplier=0,
                       allow_small_or_imprecise_dtypes=True)
        nc.gpsimd.iota(ioX[:], pattern=[[1, NX]], base=-(dxmax + 1), channel_multiplier=0,
                       allow_small_or_imprecise_dtypes=True)
        nc.gpsimd.iota(x0f[:], pattern=[[1, W]], base=0, channel_multiplier=0,
                       allow_small_or_imprecise_dtypes=True)
        nc.gpsimd.iota(y0f[:], pattern=[[128, NBLK]], base=0, channel_multiplier=1,
                       allow_small_or_imprecise_dtypes=True)

        # ---- grid accumulators (ch-inner: 0=w, 1=w*fy, 2=w*fx) ----
        # storage row = real + PAD; storage col = real + XPAD (halo cols/rows
        # only ever accumulate exact zeros)
        WS = W + 2 * XPAD
        grid = singles.tile([128, NSBLK, WS, 3], F32)
        nc.vector.memset(grid[:, :, :WS // 4], 0.0)
        nc.gpsimd.memset(grid[:, :, WS // 4:], 0.0)

        # zero operands for the per-group-segment PSUM-clearing matmul
        z_l = singles.tile([16, 128], BF16)
        z_r = singles.tile([16, 512], BF16)
        zbf = singles.tile([128, 1], BF16)
        cH = singles.tile([128, 1], F32)
        cW = singles.tile([128, 1], F32)
        nc.gpsimd.memset(z_l[:], 0.0)
        nc.gpsimd.memset(z_r[:], 0.0)
        nc.gpsimd.memset(zbf[:], 0.0)
        nc.gpsimd.memset(cH[:], float(H - 1))
        nc.gpsimd.memset(cW[:], float(W - 1))

        # ---- load inputs & pointwise prep (comb-split planes) ----
        # PSc: plane 0 = wyM (masked warped y), plane 1 = wx   (f32)
        # PSv: plane 0 = fy, plane 1 = fx                      (bf16)
        PSc = singles.tile([128, 2, NBLK, IL, NPAIRS], F32)
        PSv = singles.tile([128, 2, NBLK, IL, NPAIRS], BF16)

        with tc.tile_pool(name="inpool", bufs=1) as inpool, \
             tc.tile_pool(name="preptmp", bufs=2) as preptmp:
            in_fy = inpool.tile([128, NBLK, W], F32)
            in_fx = inpool.tile([128, NBLK, W], F32)
            for blk in range(NBLK):
                rows = min(128, H - 128 * blk)
                nc.gpsimd.dma_start(out=in_fy[:rows, blk], in_=fy_in.ap()[128 * blk:128 * blk + rows])
                nc.sync.dma_start(out=in_fx[:rows, blk], in_=fx_in.ap()[128 * blk:128 * blk + rows])
            for blk in range(NBLK):
                rows = min(128, H - 128 * blk)
                wy = preptmp.tile([128, W], F32, tag="wy")
                wx = preptmp.tile([128, W], F32, tag="wx")
                ta = preptmp.tile([128, W], F32, tag="ta")
                tb = preptmp.tile([128, W], F32, tag="tb")
                nc.vector.tensor_scalar(out=wy[:rows], in0=in_fy[:rows, blk], scalar1=dt,
                                        scalar2=y0f[:rows, blk:blk + 1], op0=Alu.mult, op1=Alu.add)
                nc.vector.scalar_tensor_tensor(out=wx[:rows], in0=in_fx[:rows, blk], scalar=dt,
                                               in1=x0f[:rows], op0=Alu.mult, op1=Alu.add)
                # mask: outside iff wy != clamp(wy) or wx != clamp(wx); exact 0/1 flags
                nc.vector.tensor_scalar(out=ta[:rows], in0=wy[:rows], scalar1=0.0,
                                        scalar2=float(H - 1), op0=Alu.max, op1=Alu.min)
                nc.vector.tensor_tensor(out=ta[:rows], in0=wy[:rows], in1=ta[:rows], op=Alu.not_equal)
                nc.vector.tensor_scalar(out=tb[:rows], in0=wx[:rows], scalar1=0.0,
                                        scalar2=float(W - 1), op0=Alu.max, op1=Alu.min)
                nc.vector.tensor_tensor(out=tb[:rows], in0=wx[:rows], in1=tb[:rows], op=Alu.not_equal)
                nc.vector.tensor_tensor(out=ta[:rows], in0=ta[:rows], in1=tb[:rows], op=Alu.add)
                # comb-split writes: dest view [rows, NPAIRS, IL] iterated (j, i), i fastest
                def split_view(t4, pl):
                    return t4[:rows, pl, blk].rearrange("p i j -> p j i")
                nc.vector.scalar_tensor_tensor(out=split_view(PSc, 0), in0=ta[:rows], scalar=BIG,
                                               in1=wy[:rows], op0=Alu.mult, op1=Alu.add)
                nc.scalar.activation(out=split_view(PSc, 1), in_=wx[:rows], func=Act.Copy)
                nc.scalar.activation(out=split_view(PSv, 0), in_=in_fy[:rows, blk], func=Act.Copy)
                nc.scalar.activation(out=split_view(PSv, 1), in_=in_fx[:rows, blk], func=Act.Copy)

        # ---- main banded splat ----
        bandp = ctx.enter_context(tc.tile_pool(name="bandp", bufs=3))
        build = ctx.enter_context(tc.tile_pool(name="build", bufs=3))
        psump = ctx.enter_context(tc.tile_pool(name="psump", bufs=8, space="PSUM"))
        tailp = ctx.enter_context(tc.tile_pool(name="tailp", bufs=2))

        for g in range(ngroups):
            gb_g = min(GB, nbands - g * GB)      # bands in this (maybe ragged) group
            ag = g * GB * BH                     # group start row
            Dg = disp_grp[g]
            ws_g = ag - (Dg + 1)                 # group window start (real row)
            rows_g = gb_g * BH + 2 * Dg + 2
            # per-segment single-bank psum tiles (8 rotating buffers);
            # uniform mapping: psum col 3*xi + c  <->  storage col IL*SP*s + xi
            psegs = [psump.tile([128, 512], F32, tag="pseg", name=f"pseg_{g}_{s}")
                     for s in range(NSEGS)]
            cleared = [False] * NSEGS

            for b in range(gb_g):
                bi = g * GB + b
                a = ag + b * BH
                Db = disp_band[bi]
                wstart = ws_g                    # real row of psum row 0
                mY = (a + BH + Db + 1) - wstart
                pfx = (a - Db - 1) - wstart      # zero lhsT cols before the window
                assert 0 <= pfx < mY <= mYmax and mY <= rows_g
                blk, p0 = divmod(a, 128)
                bandC = bandp.tile([128, 2, NPAIRS], F32, tag="bandC")   # wyM, wx
                bandV = bandp.tile([128, 2, NPAIRS], BF16, tag="bandV")  # fy, fx
                for i in range(IL):
                    nc.gpsimd.dma_start(out=bandC[BH * i:BH * (i + 1)],
                                        in_=PSc[p0:p0 + BH, :, blk, i])
                    nc.sync.dma_start(out=bandV[BH * i:BH * (i + 1)],
                                      in_=PSv[p0:p0 + BH, :, blk, i])
                for s in range(NSEGS):
                    SPs = min(SP, NPAIRS - SP * s)
                    dx = dx_map[bi][s]
                    delta = dxmax - dx   # column shift into the uniform mapping
                    XW = 2 * dx + 2 + IL
                    j0 = SP * s

                    tentY = build.tile([128, SP, mYmax], BF16, tag="tentY")
                    rhs = build.tile([128, SP, 3, XWMAX], BF16, tag="rhs")
                    fvE = build.tile([128, SP, 2, XWMAX], BF16, tag="fvE")

                    # Y tents: tentY = relu(1 - |ioY - wy|), one fused DVE pass
                    # over the natural window; ACT zero-fills the snap prefix
                    if pfx > 0:
                        nc.scalar.activation(
                            out=tentY[:, :SPs, :pfx],
                            in_=_v(zbf[:, 0:1], [[0, SPs], [0, pfx]]),
                            func=Act.Copy)
                    nc.vector._custom_dve(
                        TENT,
                        out=tentY[:, :SPs, pfx:mY],
                        in0=_v(ioY[:, (a - Db - 1) + PAD:], [[0, SPs], [1, mY - pfx]]),
                        in1=_v(bandC[:, 0, j0:j0 + SPs], [[1, SPs], [0, mY - pfx]]))
                    # X tents into rhs channel 0 (contiguous)
                    nc.vector._custom_dve(
                        TENT,
                        out=rhs[:, :SPs, 0, :XW],
                        in0=_v(ioX[:, IL * j0 + dxmax - dx:], [[IL, SPs], [1, XW]]),
                        in1=_v(bandC[:, 1, j0:j0 + SPs], [[1, SPs], [0, XW]]))
                    # expand fy/fx on the ACT engine so the DVE mul sees
                    # packed operands (keeps the 2x 16-bit DVE mode)
                    nc.scalar.activation(
                        out=fvE[:, :SPs, :, :XW],
                        in_=_v(bandV[:, 0, j0:j0 + SPs], [[1, SPs], [NPAIRS, 2], [0, XW]]),
                        func=Act.Copy)
                    # both value channels in one packed mul: ch0 read twice via
                    # a stride-0 middle dim (inner dim stays packed)
                    nc.vector.tensor_tensor(
                        out=rhs[:, :SPs, 1:3, :XW],
                        in0=_v(rhs[:, :SPs, 0, :XW], [[3 * XWMAX, SPs], [0, 2], [1, XW]]),
                        in1=fvE[:, :SPs, :, :XW],
                        op=Alu.mult)

                    if not cleared[s]:
                        # start=True zero matmul: clears the bank's has_written
                        # bits over the segment's full spill extent
                        XTg = IL * SPs + 2 * dxmax + 2
                        nc.tensor.matmul(psegs[s][:rows_g, :XTg * 3], lhsT=z_l[:, :rows_g],
                                         rhs=z_r[:, :XTg * 3], start=True, stop=False)
                        cleared[s] = True
                    last = (b == gb_g - 1)
                    for jj in range(SPs):
                        # rhs chunk read ch-inner (x outer, ch inner) to match psum
                        rhs_j = _v(rhs[:], [[1, XW], [XWMAX, 3]],
                                   extra_off=jj * 3 * XWMAX)
                        col0 = 3 * (IL * jj + delta)
                        nc.tensor.matmul(
                            psegs[s][0:mY, col0:col0 + XW * 3],
                            lhsT=tentY[:, jj, :mY],
                            rhs=rhs_j,
                            start=False, stop=(last and jj == SPs - 1))

            # spill: add group psum into the grid. Segments with the same
            # parity have disjoint storage-column windows, so each partition
            # piece needs one DVE add per parity class (plus the ragged seg).
            # storage col of psum (s, 3*xi+c) is IL*SP*s + xi (uniform map).
            y = ws_g + PAD            # storage row of psum row 0 (= ag, 32-aligned)
            assert y % 32 == 0
            s1 = min(y + rows_g, H + PAD)  # rows beyond H+PAD are exact zeros
            # engine partition-range rule: start q allows to the next
            # 64-boundary, except q%128==0 which allows all 128
            allow = lambda q: 128 - q % 128 if q % 128 == 0 else 64 - q % 64 if q % 64 else 64
            while y < s1:
                gblk, gp = divmod(y, 128)
                pr = y - (ws_g + PAD)
                ln = min(s1 - y, allow(gp), allow(pr))
                for s in range(NSEGS):
                    SPs = min(SP, NPAIRS - SP * s)
                    ext = (IL * SPs + 2 * dxmax + 2) * 3
                    c0 = IL * SP * s
                    nc.vector.tensor_tensor(
                        out=grid[gp:gp + ln, gblk, c0:c0 + ext // 3],
                        in0=_v(psegs[s][pr:pr + ln], [[1, ext]]),
                        in1=grid[gp:gp + ln, gblk, c0:c0 + ext // 3],
                        op=Alu.add)
                y += ln

            # per-block tail: normalize + store any 128-row storage block that
            # no later group touches (overlaps the remaining main loop)
            for bb_ in range(NSBLK):
                if g == min(ngroups - 1, (128 * (bb_ + 1) - 1) // (GB * BH)):
                    rec = tailp.tile([128, W], F32, tag="rec", name=f"rec{bb_}")
                    ofy = tailp.tile([128, W], F32, tag="ofy", name=f"ofy{bb_}")
                    ofx = tailp.tile([128, W], F32, tag="ofx", name=f"ofx{bb_}")
                    nc.vector.tensor_scalar(out=rec[:], in0=grid[:, bb_, XPAD:XPAD + W, 0],
                                            scalar1=EPS, scalar2=None, op0=Alu.add)
                    nc.vector.reciprocal(out=rec[:], in_=rec[:])
                    nc.vector.tensor_tensor(out=ofy[:], in0=grid[:, bb_, XPAD:XPAD + W, 1],
                                            in1=rec[:], op=Alu.mult)
                    nc.gpsimd.tensor_tensor(out=ofx[:], in0=grid[:, bb_, XPAD:XPAD + W, 2],
                                            in1=rec[:], op=Alu.mult)
                    r0 = max(0, 128 * bb_ - PAD)
                    r1 = min(H, 128 * (bb_ + 1) - PAD)
                    p0_ = r0 + PAD - 128 * bb_
                    nc.sync.dma_start(out=o_wfx.ap()[r0:r1], in_=ofx[p0_:p0_ + r1 - r0])
                    nc.gpsimd.dma_start(out=o_wfy.ap()[r0:r1], in_=ofy[p0_:p0_ + r1 - r0])


    nc.compile()
    return nc


_PROG_CACHE = {}


def _get_program(dt, disp_band, disp_grp, dx_map, H=H, W=W):
    key = (float(dt), tuple(disp_band), tuple(disp_grp),
           tuple(tuple(r) for r in dx_map), H, W)
    if key not in _PROG_CACHE:
        _PROG_CACHE[key] = _build_program(dt, disp_band, disp_grp, dx_map, H=H, W=W)
    return _PROG_CACHE[key]


def _window_params(fy, fx, dt, H=H, W=W):
    """Exact per-region displacement bounds (over all batch elements)."""
    ady = np.abs(dt) * np.abs(fy).max(axis=0)      # [H, W]
    adx = np.abs(dt) * np.abs(fx).max(axis=0)
    nbands = H // BH
    ngroups = (nbands + GB - 1) // GB
    disp_band = [max(2, int(math.ceil(float(ady[a:a + BH].max()))))
                 for a in range(0, H, BH)]
    # all groups share the global max so the group window start lands exactly
    # PAD rows early -> storage partition of psum row 0 is 32-aligned (= a_g)
    dmax = max(disp_band)
    disp_grp = [dmax for _ in range(ngroups)]
    assert len(disp_grp) == ngroups
    dx_map = []
    for a in range(0, H, BH):
        row = []
        for s in range(NSEGS):
            c0 = IL * SP * s
            c1 = min(W, IL * SP * (s + 1))
            m = float(adx[a:a + BH, c0:c1].max())
            row.append(max(2, int(math.ceil(m))))
        dx_map.append(row)
    return disp_band, disp_grp, dx_map


def kernel(flow_maps_x, flow_maps_y, i=0, tref=4):
    i = int(i)
    tref = int(tref)
    dt = float(tref - i)
    B = flow_maps_x.shape[0]
    assert B <= NCORES, f"batch {B} > {NCORES} cores not supported"
    fx = np.ascontiguousarray(flow_maps_x[:, i]).astype(np.float32)
    fy = np.ascontiguousarray(flow_maps_y[:, i]).astype(np.float32)

    disp_band, disp_grp, dx_map = _window_params(fy, fx, dt)
    nc = _get_program(dt, disp_band, disp_grp, dx_map)
    in_maps = [{"fy": fy[b], "fx": fx[b]} for b in range(B)]
    res = run_bass_kernel_spmd(nc, in_maps, list(range(B)))
    wfx = np.stack([res.results[b]["out_wfx"] for b in range(B)])[:, None]
    wfy = np.stack([res.results[b]["out_wfy"] for b in range(B)])[:, None]
    return wfx.astype(np.float32), wfy.astype(np.float32)


def _ensure_ntff_hook():
    """The agent image lacks antenv.axon_hooks; synthesize it from trn_agent_boot."""
    import types
    try:
        import antenv.axon_hooks  # noqa: F401
        return
    except ImportError:
        pass
    from trn_agent_boot.trn_boot import _ntff_profile_via_ctypes
    hook = _ntff_profile_via_ctypes("/opt/axon/libaxon_pjrt.so")
    m = types.ModuleType("antenv.axon_hooks")
    m.get_axon_ntff_profile_hook = lambda: hook
    m.set_axon_ntff_profile_hook = lambda h: None
    sys.modules["antenv.axon_hooks"] = m


def timed_run(np_inputs):
    """Run once with NTFF tracing; return HW exec time in ns (max over traced cores)."""
    _ensure_ntff_hook()
    i = int(np_inputs["i"]); tref = int(np_inputs["tref"])
    dt = float(tref - i)
    fx = np.ascontiguousarray(np_inputs["flow_maps_x"][:, i]).astype(np.float32)
    fy = np.ascontiguousarray(np_inputs["flow_maps_y"][:, i]).astype(np.float32)
    B = fx.shape[0]
    disp_band, disp_grp, dx_map = _window_params(fy, fx, dt)
    nc = _get_program(dt, disp_band, disp_grp, dx_map)
    in_maps = [{"fy": fy[b], "fx": fx[b]} for b in range(B)]
    res = run_bass_kernel_spmd(nc, in_maps, list(range(B)), trace=True)
    return res.exec_time_ns


if __name__ == "__main__":
    rng = np.random.default_rng(0)
    fmx = rng.standard_normal((8, 4, H, W), dtype=np.float32)
    fmy = rng.standard_normal((8, 4, H, W), dtype=np.float32)
    ox, oy = kernel(fmx, fmy, 0, 4)
    print(ox.shape, oy.shape, ox.dtype)
